# revision 1
# baseline (speedup 1.0000x reference)
"""Trainium2 Bass kernel for nn_Encoder (6-layer dense transformer encoder).

Sharding: pure data-parallel — batch B=8 across 8 NeuronCores, one sequence
per core. Each core runs the full 6-layer encoder on its [512, 1024] stream.

All matmuls run in float32r (TF32-like, ~1.6e-4 rel err, full PE rate at
N>=512). The reference's raw reshape [B,L,H*dh] -> [B,H,L,dh] is honored
exactly: head h covers original rows l in [32h, 32h+32); its "sequence" index
is l' = (l%32)*16 + d//64. Internally key/query slots are enumerated in
permuted "gj" order s = 32*(d//64) + (l%32), which makes the gather DMAs
contiguous; softmax sets/sums are permutation-invariant, and the pad mask uses
the same permutation. Softmax skips the max subtraction (scores are O(1);
exp of masked -1e9 underflows to exactly 0); the normalizer Z comes from an
M=2 ones-matmul over the exp tiles.

DMA-count discipline (HWDGE costs ~625ns serialized per DMA): weights load in
[128,512] groups, Q/K/V head-layout buffers fill via 32 gather-DMAs each on
the gpsimd (SWDGE) queues, V' and the context-transpose are done on the PE
(transpose mode) instead of scatter DMAs.
"""

import numpy as np

import concourse.bass as bass
import concourse.tile as tile
from concourse import bacc, mybir
from concourse.bass_utils import run_bass_kernel_spmd
from concourse.masks import make_identity

F32 = mybir.dt.float32
F32R = mybir.dt.float32r
U32 = mybir.dt.uint32
AF = mybir.ActivationFunctionType
ALU = mybir.AluOpType

B, L, D, H, DFF, V, NL = 8, 512, 1024, 16, 4096, 32000, 6
DH = D // H  # 64
LC = L // 128  # 4 l-chunks
DC = D // 128  # 8 d-chunks
FC = DFF // 128  # 32 dff-chunks
NS = D // 512  # 2 n-slices of 512
EPS = 1e-5
NEG = -1.0e9


def _positional_encoding():
    position = np.arange(L, dtype=np.float32)[:, None]
    div_term = (
        1.0
        / np.power(
            np.float32(10000.0), np.arange(0, D, 2, dtype=np.float32) / np.float32(D)
        )
    ).astype(np.float32)
    pe = np.zeros((L, D), dtype=np.float32)
    pe[:, 0::2] = np.sin(position * div_term)
    pe[:, 1::2] = np.cos(position * div_term)
    return pe


class Ctx:
    """Holds nc, pools, constants shared across the layer builder."""


def _layernorm(c, psrcs, res_tile, out_tag):
    """x_new = LN(res + psum slices). psrcs: list of (psum_ap, col) covering
    the full 1024 free dim. Returns new [128, 1024] f32 tile."""
    nc = c.nc
    t = c.pln.tile([128, D], F32, tag="lnt", name="lnt", bufs=5)
    for ap, col in psrcs:
        nc.vector.tensor_add(
            out=t[:, col : col + 512], in0=ap, in1=res_tile[:, col : col + 512]
        )
    stats = c.plns.tile([128, 2, 6], F32, tag="stats", name="stats")
    tr = t[:].rearrange("p (s f) -> p s f", s=2)
    for s in range(2):
        nc.vector.bn_stats(out=stats[:, s, :], in_=tr[:, s, :])
    mv = c.plns.tile([128, 2], F32, tag="mv", name="mv")
    nc.vector.bn_aggr(out=mv[:], in_=stats[:])
    r = c.plns.tile([128, 1], F32, tag="r", name="r")
    nc.scalar.activation(out=r[:], in_=mv[:, 1:2], func=AF.Sqrt, bias=c.eps_t[:, 0:1])
    nc.vector.reciprocal(out=r[:], in_=r[:])
    nmr = c.plns.tile([128, 1], F32, tag="nmr", name="nmr")
    nc.vector.tensor_mul(out=nmr[:], in0=mv[:, 0:1], in1=r[:])
    nc.vector.tensor_scalar_mul(out=nmr[:], in0=nmr[:], scalar1=-1.0)
    xn = c.px.tile([128, D], F32, tag=out_tag, name=out_tag)
    nc.scalar.activation(out=xn[:], in_=t[:], func=AF.Identity, bias=nmr[:], scale=r[:])
    return xn


def _make_xT(c, x_tiles, tag):
    """x normal [4x128, 1024] f32 -> transposed [8x128, 512] f32r tiles."""
    nc = c.nc
    xT = [c.pxt.tile([128, L], F32R, tag=tag, name=tag) for _ in range(DC)]
    for dc in range(DC):
        for lc in range(LC):
            p = c.psum_tr.tile([128, 128], F32, tag="trp", name="trp")
            nc.tensor.transpose(
                out=p[:],
                in_=x_tiles[lc][:, 128 * dc : 128 * (dc + 1)],
                identity=c.ident[:],
            )
            nc.vector.tensor_copy(out=xT[dc][:, 128 * lc : 128 * (lc + 1)], in_=p[:])
    return xT


def _layer(c, li, x_tiles, xT_tiles):
    nc, tc = c.nc, c.tc

    # per-layer bias tiles (all transposed-layout projections use per-partition
    # ACT bias; normal-layout outputs (Wo, W2) use the K=1 ones-matmul trick)
    bq_t = c.pbias.tile([128, DC], F32, tag="bq", name="bq")
    nc.sync.dma_start(out=bq_t[:], in_=c.bq[li].rearrange("(m p) -> p m", m=DC))
    bk_t = c.pbias.tile([128, DC], F32, tag="bk", name="bk")
    nc.sync.dma_start(out=bk_t[:], in_=c.bk[li].rearrange("(m p) -> p m", m=DC))
    bv_t = c.pbias.tile([128, DC], F32, tag="bv", name="bv")
    nc.sync.dma_start(out=bv_t[:], in_=c.bv[li].rearrange("(m p) -> p m", m=DC))
    b1_t = c.pbias.tile([128, FC], F32, tag="b1", name="b1")
    nc.sync.dma_start(out=b1_t[:], in_=c.b1[li].rearrange("(m p) -> p m", m=FC))
    bo_r = c.pbias.tile([1, D], F32R, tag="bo", name="bo", bufs=1)
    nc.sync.dma_start(out=bo_r[:], in_=c.bo[li][None, :].bitcast(F32R))
    b2_r = c.pbias.tile([1, D], F32R, tag="b2", name="b2", bufs=1)
    nc.sync.dma_start(out=b2_r[:], in_=c.b2[li][None, :].bitcast(F32R))

    ctxT = [c.pxt.tile([128, L], F32R, tag="xt", name="ctxT") for _ in range(DC)]
    x_mid = [None] * LC
    with (
        tc.tile_pool(name="pah", bufs=1) as pah,
        tc.tile_pool(name="pavp", bufs=64) as pavp,
    ):
        pavh = tc.alloc_tile_pool(name="pavh", bufs=1)
        QH = pah.tile([128, 8 * L], F32R, tag="QH", name="QH")
        KH = pah.tile([128, 8 * L], F32R, tag="KH", name="KH")
        VH = pavh.tile([128, 8 * L], F32, tag="VH", name="VH")

        def head_gather(src_tile, t, dst, cast):
            """projT-chunk tile t [128(d), 512(l)] -> head-layout buffer."""
            for gp in range(2):
                for par in range(2):
                    src = src_tile[64 * gp : 64 * gp + 64, :].rearrange(
                        "dh (p par lm) -> par dh p lm", par=2, lm=32
                    )[par]
                    if cast:
                        src = src.bitcast(F32R)
                    dsl = dst[64 * par : 64 * par + 64, :].rearrange(
                        "dh (p fb) -> dh p fb", fb=L
                    )[:, :, 64 * t + 32 * gp : 64 * t + 32 * gp + 32]
                    eng = nc.gpsimd if cast else nc.sync
                    eng.dma_start(out=dsl, in_=src)

        # ---- Q/K/V projections (transposed layout) + head gather ----
        with (
            tc.tile_pool(name="pwg", bufs=12) as pwg,
            tc.tile_pool(name="paq", bufs=2) as paq,
        ):
            for name, W_, b_t, dst, cast in (
                ("v", c.Wv, bv_t, VH, False),
                ("q", c.Wq, bq_t, QH, True),
                ("k", c.Wk, bk_t, KH, True),
            ):
                for mg in range(2):
                    wg = []
                    for k in range(DC):
                        wt = pwg.tile([128, 512], F32R, tag="wg", name="wg")
                        nc.sync.dma_start(
                            out=wt[:],
                            in_=W_[
                                li,
                                128 * k : 128 * (k + 1),
                                512 * mg : 512 * (mg + 1),
                            ].bitcast(F32R),
                        )
                        wg.append(wt)
                    for mm in range(4):
                        m = 4 * mg + mm
                        ps = c.psum_mm.tile([128, L], F32, tag="mm", name="mm")
                        for k in range(DC):
                            nc.tensor.matmul(
                                out=ps[:],
                                lhsT=wg[k][:, 128 * mm : 128 * (mm + 1)],
                                rhs=xT_tiles[k][:],
                                start=(k == 0),
                                stop=(k == DC - 1),
                            )
                        st = paq.tile([128, L], F32, tag=name + "t", name=name + "t")
                        nc.scalar.activation(
                            out=st[:],
                            in_=ps[:],
                            func=AF.Identity,
                            bias=b_t[:, m : m + 1],
                        )
                        head_gather(st, m, dst, cast)

        # ---- V' tiles via PE transpose of VH ----
        vps = {}
        for h in range(H):
            par, fb = h % 2, h // 2
            for kc in range(LC):
                tp = c.psum_tr.tile([128, DH], F32, tag="trp", name="trp")
                nc.tensor.transpose(
                    out=tp[:],
                    in_=VH[
                        64 * par : 64 * par + 64,
                        L * fb + 128 * kc : L * fb + 128 * (kc + 1),
                    ],
                    identity=c.ident[64 * par : 64 * par + 64,
                                     64 * par : 64 * par + 64],
                )
                vp = pavp.tile([128, DH + 2], F32R, tag="vp", name="vp")
                nc.vector.tensor_copy(out=vp[:, 0:DH], in_=tp[:])
                nc.vector.tensor_copy(out=vp[:, DH : DH + 2], in_=c.onescol2[:])
                vps[(h, kc)] = vp
        pavh.release()

        with (
            tc.tile_pool(name="pae", bufs=8) as pae,
            tc.tile_pool(name="pac", bufs=2) as pac,
            tc.tile_pool(name="pach", bufs=4) as pach,
        ):
            # ---- attention heads ----
            CH = [pach.tile([128, D], F32, tag="CH", name="CH") for _ in range(LC)]
            for h in range(H):
                par, fb = h % 2, h // 2

                e_tiles = []
                for kc in range(LC):
                    ps = c.psum_mm.tile([128, L], F32, tag="mm", name="mm")
                    nc.tensor.matmul(
                        out=ps[:],
                        lhsT=KH[
                            64 * par : 64 * par + 64,
                            L * fb + 128 * kc : L * fb + 128 * (kc + 1),
                        ],
                        rhs=QH[64 * par : 64 * par + 64, L * fb : L * (fb + 1)],
                        start=True,
                        stop=True,
                    )
                    et = pae.tile([128, L], F32R, tag="e", name="e")
                    nc.scalar.activation(
                        out=et[:],
                        in_=ps[:],
                        func=AF.Exp,
                        bias=c.maskb[:, kc : kc + 1],
                        scale=0.125,
                    )
                    e_tiles.append(et)

                # C'^T rows 0:64 = context^T, rows 64:66 = Z (ones cols of V')
                cps = c.psum_mm.tile([DH + 2, L], F32, tag="mm", name="mm")
                for kc in range(LC):
                    nc.tensor.matmul(
                        out=cps[:],
                        lhsT=vps[(h, kc)][:],
                        rhs=e_tiles[kc][:],
                        start=(kc == 0),
                        stop=(kc == LC - 1),
                    )
                csb = pac.tile([DH + 2, L], F32, tag="csb", name="csb")
                nc.vector.tensor_copy(out=csb[:], in_=cps[:])

                # transpose back + normalize into CH (q''-part, head-col layout)
                for qc in range(LC):
                    tp = c.psum_tr.tile([128, DH + 2], F32, tag="trp", name="trp")
                    nc.tensor.transpose(
                        out=tp[:],
                        in_=csb[:, 128 * qc : 128 * (qc + 1)],
                        identity=c.ident[: DH + 2, : DH + 2],
                    )
                    r = pac.tile([128, 1], F32, tag="rcol", name="rcol", bufs=3)
                    nc.vector.reciprocal(out=r[:], in_=tp[:, DH : DH + 1])
                    nc.vector.tensor_scalar_mul(
                        out=CH[qc][:, DH * h : DH * (h + 1)],
                        in0=tp[:, 0:DH],
                        scalar1=r[:],
                    )

            # ---- CH -> ctxT via PE 32x64 transposes into staging PSUM ----
            # ctxT tile t holds d in [128t, 128(t+1)) = g in {2t, 2t+1};
            # piece (h, qc, pg): g = 4qc + pg, src CH[qc][32pg:+32, 64h:+64]
            # -> staging psum [64, 512] for half g%2 (transpose-mode matmuls
            # must write PSUM partition 0); lower half lands in ctxT via a
            # partition-shifting SBUF->SBUF DMA.
            for qc in range(LC):
                for tt in (2 * qc, 2 * qc + 1):
                    for gp2 in range(2):
                        pg = (2 * tt + gp2) - 4 * qc
                        stg = c.psum_mm.tile([DH, L], F32, tag="mm", name="mm")
                        for h in range(H):
                            nc.tensor.transpose(
                                out=stg[:, 32 * h : 32 * (h + 1)],
                                in_=CH[qc][32 * pg : 32 * pg + 32,
                                           DH * h : DH * (h + 1)],
                                identity=c.ident[32 * pg : 32 * pg + 32,
                                                 32 * pg : 32 * pg + 32],
                                tile_position=(32 * pg, 0),
                            )
                        if gp2 == 0:
                            nc.vector.tensor_copy(out=ctxT[tt][0:DH, :], in_=stg[:])
                        else:
                            s1 = pac.tile([DH, L], F32R, tag="s1", name="s1")
                            nc.vector.tensor_copy(out=s1[:], in_=stg[:])
                            nc.sync.dma_start(
                                out=ctxT[tt][DH : 2 * DH, :], in_=s1[:]
                            )

    # ---- Wo projection + LN1 ----
    with tc.tile_pool(name="pln", bufs=1) as pln:
        c.pln = pln
        lnts = {}
        for n in range(NS):
            psl = []
            for m in range(LC):
                ps = c.psum_mm.tile([128, 512], F32, tag="mm", name="mm")
                nc.tensor.matmul(
                    out=ps[:],
                    lhsT=c.ones_t[:],
                    rhs=bo_r[:, 512 * n : 512 * (n + 1)],
                    start=True,
                    stop=False,
                )
                psl.append(ps)
            for k in range(DC):
                wt = c.pwn.tile([128, 512], F32R, tag="wn", name="wn")
                nc.sync.dma_start(
                    out=wt[:],
                    in_=c.Wo[
                        li, 128 * k : 128 * (k + 1), 512 * n : 512 * (n + 1)
                    ].bitcast(F32R),
                )
                for m in range(LC):
                    nc.tensor.matmul(
                        out=psl[m][:],
                        lhsT=ctxT[k][:, 128 * m : 128 * (m + 1)],
                        rhs=wt[:],
                        start=False,
                        stop=(k == DC - 1),
                    )
            for m in range(LC):
                lnts.setdefault(m, []).append((psl[m][:], 512 * n))
        for m in range(LC):
            x_mid[m] = _layernorm(c, lnts[m], x_tiles[m], "x")

        xmT = _make_xT(c, x_mid, "xt")

    # ---- FFN ----
    x_new = [None] * LC
    with (
        tc.tile_pool(name="pht", bufs=32) as pht,
        tc.tile_pool(name="pln", bufs=1) as pln2,
        tc.tile_pool(name="pwg", bufs=12) as pwg2,
    ):
        c.pln = pln2
        h_tiles = []
        for g4 in range(FC // 4):
            wg = []
            for k in range(DC):
                wt = pwg2.tile([128, 512], F32R, tag="wg", name="wg")
                nc.sync.dma_start(
                    out=wt[:],
                    in_=c.W1[
                        li, 128 * k : 128 * (k + 1), 512 * g4 : 512 * (g4 + 1)
                    ].bitcast(F32R),
                )
                wg.append(wt)
            for mm in range(4):
                mf = 4 * g4 + mm
                ps = c.psum_mm.tile([128, L], F32, tag="mm", name="mm")
                for k in range(DC):
                    nc.tensor.matmul(
                        out=ps[:],
                        lhsT=wg[k][:, 128 * mm : 128 * (mm + 1)],
                        rhs=xmT[k][:],
                        start=(k == 0),
                        stop=(k == DC - 1),
                    )
                ht = pht.tile([128, L], F32R, tag="ht", name="ht")
                nc.scalar.activation(
                    out=ht[:], in_=ps[:], func=AF.Gelu, bias=b1_t[:, mf : mf + 1]
                )
                h_tiles.append(ht)

        lnts = {}
        for n in range(NS):
            psl = []
            for m in range(LC):
                ps = c.psum_mm.tile([128, 512], F32, tag="mm", name="mm")
                nc.tensor.matmul(
                    out=ps[:],
                    lhsT=c.ones_t[:],
                    rhs=b2_r[:, 512 * n : 512 * (n + 1)],
                    start=True,
                    stop=False,
                )
                psl.append(ps)
            for kf in range(FC):
                wt = c.pwn.tile([128, 512], F32R, tag="wn", name="wn")
                nc.sync.dma_start(
                    out=wt[:],
                    in_=c.W2[
                        li, 128 * kf : 128 * (kf + 1), 512 * n : 512 * (n + 1)
                    ].bitcast(F32R),
                )
                for m in range(LC):
                    nc.tensor.matmul(
                        out=psl[m][:],
                        lhsT=h_tiles[kf][:, 128 * m : 128 * (m + 1)],
                        rhs=wt[:],
                        start=False,
                        stop=(kf == FC - 1),
                    )
            for m in range(LC):
                lnts.setdefault(m, []).append((psl[m][:], 512 * n))
        for m in range(LC):
            x_new[m] = _layernorm(c, lnts[m], x_mid[m], "x")

    xT_new = _make_xT(c, x_new, "xt") if li < NL - 1 else None
    return x_new, xT_new


def build_nc():
    nc = bacc.Bacc()
    c = Ctx()
    c.nc = nc

    seq_u = nc.declare_dram_parameter("seq_u", [L], U32, isOutput=False)
    seg_u = nc.declare_dram_parameter("seg_u", [L], U32, isOutput=False)
    tok_emb = nc.declare_dram_parameter("tok_emb", [V, D], F32, isOutput=False)
    seg_emb = nc.declare_dram_parameter("seg_emb", [3, D], F32, isOutput=False)
    c.Wq = nc.declare_dram_parameter("Wq", [NL, D, D], F32, isOutput=False)
    c.bq = nc.declare_dram_parameter("bq", [NL, D], F32, isOutput=False)
    c.Wk = nc.declare_dram_parameter("Wk", [NL, D, D], F32, isOutput=False)
    c.bk = nc.declare_dram_parameter("bk", [NL, D], F32, isOutput=False)
    c.Wv = nc.declare_dram_parameter("Wv", [NL, D, D], F32, isOutput=False)
    c.bv = nc.declare_dram_parameter("bv", [NL, D], F32, isOutput=False)
    c.Wo = nc.declare_dram_parameter("Wo", [NL, D, D], F32, isOutput=False)
    c.bo = nc.declare_dram_parameter("bo", [NL, D], F32, isOutput=False)
    c.W1 = nc.declare_dram_parameter("W1", [NL, D, DFF], F32, isOutput=False)
    c.b1 = nc.declare_dram_parameter("b1", [NL, DFF], F32, isOutput=False)
    c.W2 = nc.declare_dram_parameter("W2", [NL, DFF, D], F32, isOutput=False)
    c.b2 = nc.declare_dram_parameter("b2", [NL, D], F32, isOutput=False)
    out = nc.declare_dram_parameter("out", [L, D], F32, isOutput=True)

    pe_dram = nc.inline_tensor(_positional_encoding(), "pe_table")
    ones_dram = nc.inline_tensor(np.ones((1, 128), np.float32), "ones_row")
    onescol_dram = nc.inline_tensor(np.ones((128, 2), np.float32), "ones_col")

    with tile.TileContext(nc) as tc:
        c.tc = tc
        with (
            tc.tile_pool(name="pc", bufs=1) as pc,
            tc.tile_pool(name="px", bufs=8) as px,
            tc.tile_pool(name="pxt", bufs=12) as pxt,
            tc.tile_pool(name="pwn", bufs=6) as pwn,
            tc.tile_pool(name="plns", bufs=3) as plns,
            tc.tile_pool(name="pbias", bufs=2) as pbias,
            tc.tile_pool(name="psum_mm", bufs=4, space="PSUM") as psum_mm,
            tc.tile_pool(name="psum_tr", bufs=2, space="PSUM") as psum_tr,
        ):
            c.px, c.pxt, c.pwn = px, pxt, pwn
            c.plns, c.pbias = plns, pbias
            c.psum_mm, c.psum_tr = psum_mm, psum_tr

            # ---------------- constants ----------------
            ident = pc.tile([128, 128], F32, tag="ident", name="ident")
            make_identity(nc, ident[:])
            c.ident = ident
            ones_t = pc.tile([1, 128], F32R, tag="ones", name="ones")
            nc.sync.dma_start(out=ones_t[:], in_=ones_dram[:].bitcast(F32R))
            c.ones_t = ones_t
            onescol2 = pc.tile([128, 2], F32, tag="onescol2", name="onescol2")
            nc.sync.dma_start(out=onescol2[:], in_=onescol_dram[:])
            c.onescol2 = onescol2
            eps_t = pc.tile([128, 1], F32, tag="eps", name="eps")
            nc.vector.memset(eps_t[:], EPS)
            c.eps_t = eps_t

            sidx = pc.tile([128, LC], U32, tag="sidx", name="sidx")
            nc.sync.dma_start(
                out=sidx[:], in_=seq_u[:].rearrange("(cc p) -> p cc", cc=LC)
            )
            gidx = pc.tile([128, LC], U32, tag="gidx", name="gidx")
            nc.sync.dma_start(
                out=gidx[:], in_=seg_u[:].rearrange("(cc p) -> p cc", cc=LC)
            )

            # pad-mask bias in gj order: maskb[p, kc] = NEG iff seq[l']==0,
            # l' = 16*(p%32) + 4*kc + p//32
            su = pc.tile([128, LC], U32, tag="su", name="su")
            for kc in range(LC):
                for pg in range(4):
                    nc.sync.dma_start(
                        out=su[32 * pg : 32 * (pg + 1), kc : kc + 1],
                        in_=seq_u[:].rearrange(
                            "(lm kc pg) -> kc pg lm", lm=32, kc=4
                        )[kc, pg].unsqueeze(-1),
                    )
            maskb = pc.tile([128, LC], F32, tag="maskb", name="maskb")
            nc.vector.tensor_scalar(
                out=maskb[:], in0=su[:], scalar1=0, scalar2=None, op0=ALU.is_equal
            )
            nc.vector.tensor_scalar_mul(out=maskb[:], in0=maskb[:], scalar1=NEG)
            c.maskb = maskb

            # ---------------- embedding ----------------
            x_tiles = []
            with tc.tile_pool(name="pemb", bufs=2) as pemb:
                for cc in range(LC):
                    tg = pemb.tile([128, D], F32, tag="emb_tok", name="emb_tok")
                    nc.gpsimd.indirect_dma_start(
                        out=tg[:],
                        out_offset=None,
                        in_=tok_emb[:],
                        in_offset=bass.IndirectOffsetOnAxis(
                            ap=sidx[:, cc : cc + 1], axis=0
                        ),
                    )
                    sg = pemb.tile([128, D], F32, tag="emb_seg", name="emb_seg")
                    nc.gpsimd.indirect_dma_start(
                        out=sg[:],
                        out_offset=None,
                        in_=seg_emb[:],
                        in_offset=bass.IndirectOffsetOnAxis(
                            ap=gidx[:, cc : cc + 1], axis=0
                        ),
                    )
                    pt = pemb.tile([128, D], F32, tag="emb_pe", name="emb_pe")
                    nc.sync.dma_start(
                        out=pt[:], in_=pe_dram[128 * cc : 128 * (cc + 1), :]
                    )
                    xt = px.tile([128, D], F32, tag="x", name="x")
                    nc.vector.tensor_add(out=xt[:], in0=tg[:], in1=pt[:])
                    nc.vector.tensor_add(out=xt[:], in0=xt[:], in1=sg[:])
                    x_tiles.append(xt)

            xT_tiles = _make_xT(c, x_tiles, "xt")

            for li in range(NL):
                x_tiles, xT_tiles = _layer(c, li, x_tiles, xT_tiles)

            for cc in range(LC):
                nc.sync.dma_start(
                    out=out[128 * cc : 128 * (cc + 1), :], in_=x_tiles[cc][:]
                )

    nc.finalize()
    return nc


_NC_CACHE = {}


def _get_nc():
    if "nc" not in _NC_CACHE:
        _NC_CACHE["nc"] = build_nc()
    return _NC_CACHE["nc"]


def _in_maps(inputs):
    sequence = np.asarray(inputs["sequence"])
    segment_label = np.asarray(inputs["segment_label"])
    arrs = {
        k: np.ascontiguousarray(np.asarray(inputs[k], dtype=np.float32))
        for k in (
            "tok_emb", "seg_emb", "Wq", "bq", "Wk", "bk", "Wv", "bv",
            "Wo", "bo", "W1", "b1", "W2", "b2",
        )
    }
    maps = []
    for cc in range(B):
        m = dict(arrs)
        m["seq_u"] = np.ascontiguousarray(sequence[cc].astype(np.uint32))
        m["seg_u"] = np.ascontiguousarray(segment_label[cc].astype(np.uint32))
        maps.append(m)
    return maps


def kernel(**inputs):
    nc = _get_nc()
    res = run_bass_kernel_spmd(nc, _in_maps(inputs), core_ids=list(range(B)))
    return np.stack([r["out"] for r in res.results]).astype(np.float32)


def run_traced(**inputs):
    """Like kernel() but with trace=True; returns (output, BassKernelResults)."""
    nc = _get_nc()
    res = run_bass_kernel_spmd(
        nc, _in_maps(inputs), core_ids=list(range(B)), trace=True
    )
    out = np.stack([r["out"] for r in res.results]).astype(np.float32)
    return out, res

